# revision 1
# baseline (speedup 1.0000x reference)
"""TRN2 Bass kernel for nn_Blur: depthwise 4x4 FIR blur (stylegan2 upfirdn2d).

out[n,c,h,w] = sum_{i,j} wflip[i,j] * x[n,c,h+i-1,w+j-1]   (zero-padded)

v5 strategy (per NeuronCore, 8-way data parallel over the 512 (n,c) images):
  - bf16 on the wire both ways (host converts); ~1e-3 rel err vs the 2e-2
    gate.  Host pads each image to [514, 516]: leading zero row = top pad,
    zero cols 0/514/515 = horizontal pads.
  - DMA count minimized (the DMA pipe serializes per transfer): per image
    ONE 4-segment load (all row-blocks, overlapping windows) and ONE
    4-segment store; per 8-image group ONE packed tail load [14, 8*516]
    and ONE packed tail store [12, 8*512].  18 DMAs per group total.
  - Horizontal taps pair-folded on DVE (y1 = x<<0 + x<<3, y2 = x<<1+x<<2),
    vertical profile via banded stationary matrices: 8 matmuls per image
    for rows 0..499.  Tails run unfolded (4 taps) per image from the
    packed tail tile: 4 tiny matmuls per image.
  - PSUM->SBUF copies all on the Scalar engine; Vector does only the
    folds; GpSimd only issues DMAs.

Self-contained: hardcodes shapes from the problem spec.
"""

import numpy as np
import ml_dtypes
from contextlib import ExitStack, nullcontext

from bass_rust import AP as RustAP
import concourse.bacc as bacc
import concourse.mybir as mybir
import concourse.tile as tile
from concourse.bass_utils import run_bass_kernel_spmd

BF16 = ml_dtypes.bfloat16

N_IMG, C, H, W = 4, 128, 513, 513
OH = OW = 512
NCORES = 8
IPC = (N_IMG * C) // NCORES  # 64 images per core

NB = 4          # row blocks per image
BM = 125        # output rows per block
XW = 516        # padded row width (1 left zero + 513 data + 2 right zeros)
SH = 514        # slab rows per image (1 zero row + 513 data rows)
TPACK = 8
T_R0, T_K, T_H0, T_BM = 499, 14, 500, 12

XBUFS = 8
YBUFS = 5
OBUFS = 5

TRACE = False
LAST_RESULTS = None

_CACHE = {}


def _split_separable(kernel):
    """kernel (4,4) -> (u[4], c1, c2): wflip[i,j] = u[i]*v[j] with v the
    symmetric horizontal profile, c1 = v[0] = v[3], c2 = v[1] = v[2]."""
    wf = np.flip(np.asarray(kernel, dtype=np.float64), (0, 1))
    s = wf.sum()
    u = wf.sum(axis=1)
    v = wf.sum(axis=0) / s
    assert np.allclose(np.outer(u, v), wf, atol=1e-6), "kernel not separable"
    assert np.allclose(v[0], v[3]) and np.allclose(v[1], v[2]), (
        "horizontal profile not symmetric"
    )
    return u, v[0], v[1]


def _make_bands_np(kernel):
    """Pack band matrices into one [128, 6*128] f32 array.

    col 0*128: V1 [128,125] banded vertical profile * c1 (outer h taps)
    col 1*128: V2 [128,125] banded vertical profile * c2 (inner h taps)
    col (2+j)*128: TBD_j [112,96] block-diagonal tail band for h tap j.
    Main semantics: partition k of block b holds input row 125b-1+k
    (row -1 = per-image zero pad row), band[k, m] = u[k-m] * c.
    Tail: partition g*14+t holds image g's input row 499+t.
    """
    u, c1, c2 = _split_separable(kernel)
    wflip = np.flip(np.asarray(kernel, dtype=np.float64), (0, 1))
    bands = np.zeros((128, 6 * 128), dtype=np.float32)

    k_idx = np.arange(128)[:, None]
    m_idx = np.arange(BM)[None, :]
    i_idx = k_idx - m_idx
    valid = (i_idx >= 0) & (i_idx < 4)
    vband = np.where(valid, u[np.clip(i_idx, 0, 3)], 0.0)
    bands[:, 0 * 128 : 0 * 128 + BM] = (vband * c1).astype(np.float32)
    bands[:, 1 * 128 : 1 * 128 + BM] = (vband * c2).astype(np.float32)

    t_idx = np.arange(T_K)[:, None]
    m_idx = np.arange(T_BM)[None, :]
    i_idx = t_idx - m_idx  # (499+t) - (500+m) + 1
    tvalid = (i_idx >= 0) & (i_idx < 4)
    for j in range(4):
        blk = np.where(tvalid, wflip[np.clip(i_idx, 0, 3), j], 0.0).astype(np.float32)
        for g in range(TPACK):
            for m in range(T_BM):
                # out partition m*TPACK+g (row-major) <- image g rows 499+t
                bands[
                    g * T_K : (g + 1) * T_K,
                    (2 + j) * 128 + m * TPACK + g,
                ] = blk[:, m]
    return bands


def _build(ipc=IPC, reps=1):
    f32 = mybir.dt.float32
    bf16 = mybir.dt.bfloat16
    nc = bacc.Bacc("TRN2", target_bir_lowering=False, debug=False)

    x_d = nc.dram_tensor("x", [ipc * SH, XW], bf16, kind="ExternalInput")
    bands_d = nc.dram_tensor("bands", [128, 6 * 128], bf16, kind="ExternalInput").ap()
    out_d = nc.dram_tensor("out", [ipc, OH, OW], bf16, kind="ExternalOutput")

    # Dedicated rings — a store waiting on compute must not
    # head-of-line-block the next load.  Loads ride the Pool SWDGE so the
    # HWDGE descriptor-gen device only carries the stores.
    def dma_load(out, in_):
        return nc.gpsimd.dma_start(out, in_)

    def dma_store(out, in_):
        return nc.scalar.dma_start(out, in_)

    def dma_tail(out, in_):
        return nc.gpsimd.dma_start(out, in_)

    def dma_tailstore(out, in_):
        return nc.sync.dma_start(out, in_)

    with tile.TileContext(nc) as tc, ExitStack() as ctx:
        cpool = ctx.enter_context(tc.tile_pool(name="const", bufs=1))
        xpool = ctx.enter_context(tc.tile_pool(name="x", bufs=XBUFS))
        ypool = ctx.enter_context(tc.tile_pool(name="y", bufs=YBUFS))
        tpool = ctx.enter_context(tc.tile_pool(name="xtail", bufs=3))
        opool = ctx.enter_context(tc.tile_pool(name="o", bufs=OBUFS))
        topool = ctx.enter_context(tc.tile_pool(name="ot", bufs=3))
        pspool = ctx.enter_context(tc.tile_pool(name="ps", bufs=8, space="PSUM"))

        bands_sb = cpool.tile([128, 6 * 128], bf16, tag="br")
        nc.sync.dma_start(bands_sb[:], bands_d[:])

        it = 0

        loop_cm = tc.For_i(0, reps, 1) if reps > 1 else nullcontext()
        with loop_cm:
          for grp in range(ipc // TPACK):
            imgs = range(grp * TPACK, (grp + 1) * TPACK)

            # ---- ONE packed tail load: [112, 516] (partition g*14+t)
            xtail = tpool.tile([128, XW], bf16, tag="xt", name=f"xtl{grp}")
            t_in = RustAP(
                x_d,
                (grp * TPACK * SH + 1 + T_R0) * XW,
                [[SH * XW, TPACK], [XW, T_K], [1, XW]],
            )
            dma_tail(xtail[0 : TPACK * T_K, :], t_in)


            for g8, img in enumerate(imgs):
                # ---- ONE 4-segment load (overlapping row-block windows)
                xt = xpool.tile([128, NB * XW], bf16, tag="xb", name=f"xb{it}")
                x_in = RustAP(
                    x_d,
                    img * SH * XW,
                    [[XW, 128], [BM * XW, NB], [1, XW]],
                )
                x_out = xt[0:128, :].rearrange("p (b w) -> p b w", b=NB)
                dma_load(x_out, x_in)

                if True:
                    # ---- horizontal pair-fold on DVE
                    y = ypool.tile([128, 2 * NB * OW], bf16, tag="y", name=f"y{it}")
                    for b in range(NB):
                        bX = b * XW
                        nc.vector.tensor_add(
                            y[0:128, (0 * NB + b) * OW : (0 * NB + b + 1) * OW],
                            xt[0:128, bX + 0 : bX + 0 + OW],
                            xt[0:128, bX + 3 : bX + 3 + OW],
                        )
                        nc.vector.tensor_add(
                            y[0:128, (1 * NB + b) * OW : (1 * NB + b + 1) * OW],
                            xt[0:128, bX + 1 : bX + 1 + OW],
                            xt[0:128, bX + 2 : bX + 2 + OW],
                        )

                    # ---- 8 matmuls (2 per block), copies on ACT
                    ot = opool.tile([128, NB * OW], bf16, tag="ob", name=f"ob{it}")
                    for b in range(NB):
                        p = pspool.tile([128, OW], f32, tag="p", name=f"p{it}_{b}")
                        for s in range(2):
                            nc.tensor.matmul(
                                p[0:BM, :],
                                bands_sb[0:128, s * 128 : s * 128 + BM],
                                y[0:128, (s * NB + b) * OW : (s * NB + b + 1) * OW],
                                start=(s == 0),
                                stop=(s == 1),
                            )
                        nc.scalar.copy(ot[0:BM, b * OW : (b + 1) * OW], p[0:BM, :])


                # ---- ONE 4-segment store (rows 0..499)
                if True:
                    o_out = RustAP(
                        out_d,
                        img * OH * OW,
                        [[OW, BM], [BM * OW, NB], [1, OW]],
                    )
                    o_in = ot[0:BM, :].rearrange("p (b w) -> p b w", b=NB)
                    dma_store(o_out, o_in)
                it += 1

            # ---- tail: 4 block-diagonal matmuls + one copy + one store
            TP = TPACK * T_K
            TB = TPACK * T_BM
            if True:
                pt = pspool.tile([128, OW], f32, tag="p", name=f"pt{grp}")
                for j in range(4):
                    nc.tensor.matmul(
                        pt[0:TB, :],
                        bands_sb[0:TP, (2 + j) * 128 : (2 + j) * 128 + TB],
                        xtail[0:TP, j : j + OW],
                        start=(j == 0),
                        stop=(j == 3),
                    )
                tto = topool.tile([128, OW], bf16, tag="to", name=f"tto{grp}")
                nc.scalar.copy(tto[0:TB, :], pt[0:TB, :])
            if True:
                to_out = RustAP(
                    out_d,
                    (grp * TPACK * OH + T_H0) * OW,
                    [[OW, T_BM], [OH * OW, TPACK], [1, OW]],
                )
                dma_tailstore(to_out, tto[0:TB, :])

    nc.compile()
    return nc


def _pad_input(x_imgs):
    """[n, 513, 513] f32 -> [n*514, 516] bf16 slab with per-image zero pad row
    and zero pad cols."""
    n = x_imgs.shape[0]
    P = np.zeros((n * SH, XW), dtype=BF16)
    P3 = P.reshape(n, SH, XW)
    P3[:, 1:, 1:514] = x_imgs.astype(BF16)
    return P


def kernel(input, kernel):
    global LAST_RESULTS
    x = np.ascontiguousarray(np.asarray(input, dtype=np.float32))
    kern = np.asarray(kernel, dtype=np.float32)
    assert x.shape == (N_IMG, C, H, W), x.shape

    if "nc" not in _CACHE:
        _CACHE["nc"] = _build()
    nc = _CACHE["nc"]

    bands = _make_bands_np(kern).astype(BF16)
    P = _pad_input(x.reshape(N_IMG * C, H, W))
    rows_per_core = IPC * SH
    in_maps = [
        {"x": P[k * rows_per_core : (k + 1) * rows_per_core], "bands": bands}
        for k in range(NCORES)
    ]
    res = run_bass_kernel_spmd(nc, in_maps, list(range(NCORES)), trace=TRACE)
    LAST_RESULTS = res

    out = np.concatenate([res.results[k]["out"] for k in range(NCORES)], axis=0)
    return out.astype(np.float32).reshape(N_IMG, C, OH, OW)

